# revision 15
# baseline (speedup 1.0000x reference)
"""Trainium2 Bass kernel for nn_ExpandEvecs.

Computes, for evecs [B=4, C=1, N=1024, K=16]:
    cube[b,l] = V[:, :l+1] @ V[:, :l+1]^T   (Gram expansion per level)
    -> [B, K, N, N] fp32 (cumsum of per-eigvec outer products over l).

Sharding: 8 cores = 4 batches x 2 row-halves; core c (b=c//2, h=c%2)
produces all 16 levels for its 512-row half. No communication.

Why this structure (per core, 8.4M output elements):
  - The PE streams 128 output elems/cycle at a sustained 1.2 GHz
    -> 54.7 us if it produced everything. Only DVE (0.96 GHz) and ACT
    (1.2 GHz) can read PSUM (~105 G elem/s each measured), and DMA can
    never touch PSUM, so every PE-produced element costs one DVE/ACT
    op too.  GpSimd is idle in a pure-PE design.
  - The cumsum structure lets GpSimd produce whole levels in SBUF
    without PSUM: out_l = out_{l-1} + u_l[p] * w_l[j], one fused
    scalar_tensor_tensor per element, where w_l is v_l replicated
    across partitions (made by a tiny ones-matmul on the PE + one
    evac) and u_l is a per-partition scalar from the host.
  - Split: PE computes level 7 FIRST (the chain seed, evacuated twice:
    int8 output + fp16 seed), then levels 0-6 (int8 out) and the 8
    replicas; GpSimd chains levels 8-15 in fp16 (stored as fp16, 2x
    DMA bytes but DMA has slack).  Engine budgets land ~27-34 us each
    (PE ~34, GpSimd ~30, ACT+DVE ~27, DMA ~33.5) vs 54.7+ for any
    single-engine-bound design.

Precision (gate 2e-2; simulated 8.3e-3 worst-case):
  - fp8 split matmuls: V = A + B, A = e4m3(V), B = e4m3(V-A);
    V V^T ~= A A^T + A B^T + B A^T. 4 stacked rows per eigvec k
    (lhs A,B,A,0 / rhs A,A,B,0) so DoubleRow pairs never straddle k.
    (DoubleRow is throughput-neutral here but keeps the operand format
    ready; correctness verified on HW at rel 4.4e-3.)
  - int8 levels use a per-(level,row) scale from the Cauchy-Schwarz
    bound ||v_i||_l * max_j ||v_j||_l (host-computed, 2% margin),
    applied during evacuation; host dequantizes.  fp16 levels are
    plain casts.
"""

import numpy as np
import ml_dtypes

import concourse.mybir as mybir
from concourse import bacc, bass
from concourse.tile import TileContext
from concourse.bass_utils import run_bass_kernel_spmd

B, C, N, K = 4, 1, 1024, 16
NCORES = 8
HALF = N // 2          # rows per core
KI = 2 * K             # DoubleRow pair-partitions at the deepest level
NPE = 9                # levels 0..NPE-1 on the PE (int8 out)
NDIR = K - NPE         # levels NPE..15 chained in SBUF (fp16 out)
SEED = NPE - 1         # chain seed level (computed first on the PE)
# chain level owners: G = gpsimd (tensor_scalar delta + tensor_tensor add,
# 2 ops), D = DVE (fused scalar_tensor_tensor, 1 op; walrus rejects STT on
# the Pool engine)
CHAIN = "GDGDGDG"

F32 = mybir.dt.float32
FP16 = mybir.dt.float16
FP8 = mybir.dt.float8e4
I8 = mybir.dt.int8
FP8_NP = ml_dtypes.float8_e4m3

_nc_cache = None


def _build():
    nc = bacc.Bacc(None, target_bir_lowering=False)
    t_d = nc.declare_dram_parameter("t", [KI, 2, N], FP8, isOutput=False)
    tl_d = nc.declare_dram_parameter("tl", [KI, 2, HALF], FP8, isOutput=False)
    sc_d = nc.declare_dram_parameter("sc", [128, NPE * 4], F32, isOutput=False)
    usc_d = nc.declare_dram_parameter("usc", [128, NDIR * 4], F32, isOutput=False)
    ones_d = nc.declare_dram_parameter("ones", [2, 128], FP8, isOutput=False)
    # replica sources at base partition 0 (matmul operands must be
    # tile-grid aligned): row 0 = A_l, row 1 = B_l for direct levels
    rep_d = nc.declare_dram_parameter("rep", [2, NDIR * N], FP8, isOutput=False)
    oi8_d = nc.declare_dram_parameter("oi8", [NPE, HALF, N], I8, isOutput=True)
    of16_d = nc.declare_dram_parameter("of16", [NDIR, HALF, N], FP16, isOutput=True)

    DR = mybir.MatmulPerfMode.DoubleRow
    COPY = mybir.ActivationFunctionType.Copy
    MUL = mybir.AluOpType.mult
    ADD = mybir.AluOpType.add

    evac_idx = [0]

    with TileContext(nc) as tc:
        with (
            tc.tile_pool(name="vpool", bufs=1) as vpool,
            tc.tile_pool(name="sti", bufs=3) as sti,
            tc.tile_pool(name="stf", bufs=3) as stf,
            tc.tile_pool(name="wrp", bufs=3) as wrp,
            tc.tile_pool(name="dlt", bufs=2) as dlt,
            tc.tile_pool(name="psum", bufs=4, space=bass.MemorySpace.PSUM) as psum,
        ):
            t = vpool.tile([KI, 2, N], FP8)
            tl = vpool.tile([KI, 2, HALF], FP8)
            sc = vpool.tile([128, NPE * 4], F32)
            usc = vpool.tile([128, NDIR * 4], F32)
            ones = vpool.tile([2, 128], FP8)
            rep = vpool.tile([2, NDIR * N], FP8)
            nc.sync.dma_start(out=tl[:], in_=tl_d[:])
            nc.scalar.dma_start(out=t[:], in_=t_d[:])
            nc.sync.dma_start(out=sc[:], in_=sc_d[:])
            nc.scalar.dma_start(out=usc[:], in_=usc_d[:])
            nc.sync.dma_start(out=ones[:], in_=ones_d[:])
            nc.scalar.dma_start(out=rep[:], in_=rep_d[:])

            # partition p of a psum/stage tile holds DRAM row 4p+r -> 4 KiB
            # (int8) / 8 KiB (fp16) contiguous store runs per partition.
            tlv = tl.rearrange("k o (m r) -> k o m r", m=128, r=4)

            def evac(dst, src, s_ap=None):
                # weighted ACT/DVE split (both can read PSUM; DVE also
                # carries the STT chain levels, so ACT takes ~2/3)
                evac_idx[0] += 13
                if evac_idx[0] >= 20:
                    evac_idx[0] -= 20
                    nc.scalar.activation(dst, src, COPY,
                                         scale=1.0 if s_ap is None else s_ap)
                elif s_ap is None:
                    nc.vector.tensor_copy(dst, src)
                else:
                    nc.vector.tensor_scalar(dst, src, s_ap, None, MUL)

            def pe_level(lvl, st, seed_st=None):
                ki = 2 * (lvl + 1)
                for r in range(4):
                    ps = psum.tile([128, N], F32, tag="ps")
                    for j in range(2):
                        nc.tensor.matmul(
                            ps[:, j * 512:(j + 1) * 512],
                            lhsT=tlv[:ki, :, :, r],
                            rhs=t[:ki, :, j * 512:(j + 1) * 512],
                            start=True, stop=True, perf_mode=DR,
                        )
                    evac(st[:, r, :], ps[:], sc[:, 4 * lvl + r:4 * lvl + r + 1])
                    if seed_st is not None:
                        evac(seed_st[:, r, :], ps[:])
                nc.sync.dma_start(
                    out=oi8_d[lvl].rearrange("(p r) f -> p r f", p=128),
                    in_=st[:, :, :],
                )

            def replica(lvl):
                # w_rep[p, j] = v_lvl[j] for all p, via ones^T @ (a_l + b_l)
                d = lvl - NPE
                ps = psum.tile([128, N], F32, tag="ps")
                for j in range(2):
                    nc.tensor.matmul(
                        ps[:, j * 512:(j + 1) * 512],
                        lhsT=ones[:, :],
                        rhs=rep[:, d * N + j * 512:d * N + (j + 1) * 512],
                        start=True, stop=True,
                    )
                w = wrp.tile([128, N], FP16, tag="w")
                evac(w[:], ps[:])
                return w

            # seed level first so the GpSimd chain can start early
            st_seed = stf.tile([128, 4, N], FP16, tag="stf")
            st7 = sti.tile([128, 4, N], I8, tag="sti")
            pe_level(SEED, st7, seed_st=st_seed)

            wreps = {}
            for lvl in range(NPE, K):
                wreps[lvl] = replica(lvl)

            # interleave: chain level (GpSimd 2-op or DVE fused STT), then
            # a PE int8 level
            prev = st_seed
            pe_todo = list(range(SEED)) + [None] * max(0, NDIR - SEED)
            for i, lvl in enumerate(range(NPE, K)):
                stg = stf.tile([128, 4, N], FP16, tag="stf")
                w = wreps[lvl]
                d = lvl - NPE
                for r in range(4):
                    u_ap = usc[:, 4 * d + r:4 * d + r + 1]
                    if CHAIN[d] == "G":
                        dl = dlt.tile([128, N], FP16, tag="dl")
                        nc.gpsimd.tensor_scalar(dl[:], w[:], u_ap, None, MUL)
                        nc.gpsimd.tensor_tensor(
                            stg[:, r, :], dl[:], prev[:, r, :], ADD)
                    else:
                        nc.vector.scalar_tensor_tensor(
                            stg[:, r, :], w[:], u_ap, prev[:, r, :], MUL, ADD)
                nc.sync.dma_start(
                    out=of16_d[d].rearrange("(p r) f -> p r f", p=128),
                    in_=stg[:, :, :],
                )
                prev = stg
                if i < len(pe_todo) and pe_todo[i] is not None:
                    st = sti.tile([128, 4, N], I8, tag="sti")
                    pe_level(pe_todo[i], st)
            for lvl in range(NDIR, SEED):
                st = sti.tile([128, 4, N], I8, tag="sti")
                pe_level(lvl, st)

    nc.compile()
    return nc


def _get_nc():
    global _nc_cache
    if _nc_cache is None:
        _nc_cache = _build()
    return _nc_cache


def _prepare_in_maps(evecs: np.ndarray):
    in_maps = []
    bounds = []
    for c in range(NCORES):
        b, h = divmod(c, 2)
        vt = np.ascontiguousarray(evecs[b, 0].T, dtype=np.float32)  # [K, N]
        a32 = vt.astype(FP8_NP).astype(np.float32)
        b32 = (vt - a32).astype(FP8_NP).astype(np.float32)
        sl = slice(h * HALF, (h + 1) * HALF)

        rhs = np.zeros((4 * K, N), dtype=np.float32)
        rhs[0::4] = a32
        rhs[1::4] = a32
        rhs[2::4] = b32
        lhs = np.zeros((4 * K, HALF), dtype=np.float32)
        lhs[0::4] = a32[:, sl]
        lhs[1::4] = b32[:, sl]
        lhs[2::4] = a32[:, sl]
        t = rhs.reshape(KI, 2, N).astype(FP8_NP)
        tl = lhs.reshape(KI, 2, HALF).astype(FP8_NP)

        # int8 scales for PE levels from the Cauchy-Schwarz bound
        cn = np.sqrt(np.cumsum(vt * vt, axis=0))          # [K, N]
        maxn = cn.max(axis=1)                             # [K]
        bound = cn[:NPE, sl] * maxn[:NPE, None] * 1.02    # [NPE, HALF]
        s = (127.0 / bound).astype(np.float32)
        sc = np.ascontiguousarray(
            s.reshape(NPE, 128, 4).transpose(1, 0, 2).reshape(128, NPE * 4)
        )
        # per-partition chain scalars u for direct levels
        u = vt[NPE:, sl]                                  # [NDIR, HALF]
        usc = np.ascontiguousarray(
            u.reshape(NDIR, 128, 4).transpose(1, 0, 2).reshape(128, NDIR * 4)
        ).astype(np.float32)
        ones = np.ones((2, 128), dtype=FP8_NP)
        rep = np.empty((2, NDIR * N), dtype=FP8_NP)
        rep[0] = a32[NPE:].reshape(-1)
        rep[1] = b32[NPE:].reshape(-1)
        in_maps.append({"t": t, "tl": tl, "sc": sc, "usc": usc,
                        "ones": ones, "rep": rep})
        bounds.append(bound)
    return in_maps, bounds


def _assemble(results, bounds) -> np.ndarray:
    out = np.empty((B, K, N, N), dtype=np.float32)
    for c in range(NCORES):
        b, h = divmod(c, 2)
        rs = slice(h * HALF, (h + 1) * HALF)
        q = results[c]["oi8"].astype(np.float32)          # [NPE, HALF, N]
        q *= (bounds[c] / 127.0)[:, :, None]
        out[b, :NPE, rs, :] = q
        out[b, NPE:, rs, :] = results[c]["of16"]          # fp16 -> fp32
    return out.reshape(B, K * C, N, N)


def kernel(evecs) -> np.ndarray:
    evecs = np.asarray(evecs, dtype=np.float32)
    assert evecs.shape == (B, C, N, K), evecs.shape
    nc = _get_nc()
    in_maps, bounds = _prepare_in_maps(evecs)
    last_err = None
    for _attempt in range(3):
        try:
            r = run_bass_kernel_spmd(nc, in_maps, list(range(NCORES)))
            return _assemble(r.results, bounds)
        except Exception as e:  # transient NRT/device hiccups: retry
            last_err = e
    raise last_err


# revision 16
# speedup vs baseline: 3.4469x; 3.4469x over previous
"""Trainium2 Bass kernel for nn_ExpandEvecs.

Computes, for evecs [B=4, C=1, N=1024, K=16]:
    cube[b,l] = V[:, :l+1] @ V[:, :l+1]^T   (Gram expansion per level)
    -> [B, K, N, N] fp32 (cumsum of per-eigvec outer products over l).

Sharding: 8 cores = 4 batches x 2 row-halves; core c (b=c//2, h=c%2)
produces all 16 levels for its 512-row half. No communication.

Performance model (per core, 8.4M output elements; all rates HW-measured):
  - The PE streams one 512-column matmul per 427 ns (1.2 GHz sustained;
    the 2.4 GHz p-state needs 3 us of gapless execution, unreachable
    when PSUM drain paces the PE) -> 128 matmuls = 54.7 us. This is the
    kernel's floor: the PE is the only engine that can produce outer
    products at rate (GpSimd tensor ops measured 2.1-15 us per 131K
    elems, DVE fused STT 9.4 us -> offload designs all lose).
  - PSUM evacuation: only ACT (1.2 GHz) and DVE (0.96 GHz) have PSUM
    ports. Whole [128, 2048] ops (one 4-bank PSUM tile) alternate
    between them ~53:47 -> ~35 us in parallel, under the PE floor.
  - int8 output (8.4 MB -> ~23.5 us at the ~358 GB/s per-core HBM
    limit) keeps DMA far off the critical path; fp32 would be 94 us.

Precision (gate 2e-2; simulated end-to-end 4.5e-3):
  - fp8 split matmuls: V = A + B, A = e4m3(V), B = e4m3(V-A);
    V V^T ~= A A^T + A B^T + B A^T (dropped B B^T ~2^-8). 4 stacked
    rows per eigvec k (lhs A,B,A,0 / rhs A,A,B,0) so DoubleRow pairs
    never straddle a k boundary.
  - int8 scale per (level, partition) from the Cauchy-Schwarz bound
    max over the partition's 4 interleaved rows of
    ||v_i||_l * max_j ||v_j||_l (host-computed, 2% margin), applied
    during evacuation (ACT activation scale= / DVE tensor_scalar,
    which round to nearest). Host dequantizes during the unshard.
  - Row-pair interleave: partition p holds DRAM rows 4p..4p+3, giving
    4 KiB contiguous int8 store runs per partition.
"""

import numpy as np
import ml_dtypes

import concourse.mybir as mybir
from concourse import bacc, bass
from concourse.tile import TileContext
from concourse.bass_utils import run_bass_kernel_spmd

B, C, N, K = 4, 1, 1024, 16
NCORES = 8
HALF = N // 2          # rows per core
KI = 2 * K             # DoubleRow pair-partitions at the deepest level

F32 = mybir.dt.float32
FP8 = mybir.dt.float8e4
I8 = mybir.dt.int8
FP8_NP = ml_dtypes.float8_e4m3

_nc_cache = None


def _build():
    nc = bacc.Bacc(None, target_bir_lowering=False)
    t_d = nc.declare_dram_parameter("t", [KI, 2, N], FP8, isOutput=False)
    tl_d = nc.declare_dram_parameter("tl", [KI, 2, HALF], FP8, isOutput=False)
    sc_d = nc.declare_dram_parameter("sc", [128, K], F32, isOutput=False)
    out_d = nc.declare_dram_parameter("out", [K, HALF, N], I8, isOutput=True)

    DR = mybir.MatmulPerfMode.DoubleRow
    COPY = mybir.ActivationFunctionType.Copy
    MUL = mybir.AluOpType.mult
    acc = [0]

    with TileContext(nc) as tc:
        with (
            tc.tile_pool(name="vpool", bufs=1) as vpool,
            tc.tile_pool(name="stage", bufs=3) as stage,
            tc.tile_pool(name="psum", bufs=2, space=bass.MemorySpace.PSUM) as psum,
        ):
            t = vpool.tile([KI, 2, N], FP8)
            tl = vpool.tile([KI, 2, HALF], FP8)
            sc = vpool.tile([128, K], F32)
            t0 = vpool.tile([4, 2, N], FP8)
            tl0 = vpool.tile([4, 2, HALF], FP8)
            # mini slices for levels 0-1 land first and unblock the PE
            # ~1.5us earlier than the full stacks; two HWDGE rings
            nc.sync.dma_start(out=tl0[:], in_=tl_d[:4])
            nc.scalar.dma_start(out=t0[:], in_=t_d[:4])
            nc.sync.dma_start(out=sc[:], in_=sc_d[:])
            nc.sync.dma_start(out=tl[:], in_=tl_d[:])
            nc.scalar.dma_start(out=t[:], in_=t_d[:])

            tlv = tl.rearrange("k o (m r) -> k o m r", m=128, r=4)
            tlv0 = tl0.rearrange("k o (m r) -> k o m r", m=128, r=4)

            for lvl in range(K):
                ki = 2 * (lvl + 1)
                lhs_all, rhs_all = (tlv0, t0) if lvl < 2 else (tlv, t)
                st = stage.tile([128, 4, N], I8, tag="st")
                s_ap = sc[:, lvl:lvl + 1]
                for q in range(2):          # r-pairs (0,1) and (2,3)
                    ps = psum.tile([128, 2, N], F32, tag="ps")  # 4 banks
                    for rr in range(2):
                        for j in range(2):
                            nc.tensor.matmul(
                                ps[:, rr, j * 512:(j + 1) * 512],
                                lhsT=lhs_all[:ki, :, :, 2 * q + rr],
                                rhs=rhs_all[:ki, :, j * 512:(j + 1) * 512],
                                start=True, stop=True, perf_mode=DR,
                            )
                    # one whole-tile [128, 2048] scale+cast evacuation,
                    # alternating ACT:DVE ~ 8:7 (their op-rate ratio)
                    acc[0] += 8
                    if acc[0] >= 15:
                        acc[0] -= 15
                        nc.scalar.activation(st[:, 2 * q:2 * q + 2, :], ps[:],
                                             COPY, scale=s_ap)
                    else:
                        nc.vector.tensor_scalar(st[:, 2 * q:2 * q + 2, :],
                                                ps[:], s_ap, None, MUL)
                nc.sync.dma_start(
                    out=out_d[lvl].rearrange("(p r) f -> p r f", p=128),
                    in_=st[:, :, :],
                )

    nc.compile()
    return nc


def _get_nc():
    global _nc_cache
    if _nc_cache is None:
        _nc_cache = _build()
    return _nc_cache


def _prepare_in_maps(evecs: np.ndarray):
    in_maps = []
    bounds = []
    for c in range(NCORES):
        b, h = divmod(c, 2)
        vt = np.ascontiguousarray(evecs[b, 0].T, dtype=np.float32)  # [K, N]
        a32 = vt.astype(FP8_NP).astype(np.float32)
        b32 = (vt - a32).astype(FP8_NP).astype(np.float32)
        sl = slice(h * HALF, (h + 1) * HALF)

        rhs = np.zeros((4 * K, N), dtype=np.float32)
        rhs[0::4] = a32
        rhs[1::4] = a32
        rhs[2::4] = b32
        lhs = np.zeros((4 * K, HALF), dtype=np.float32)
        lhs[0::4] = a32[:, sl]
        lhs[1::4] = b32[:, sl]
        lhs[2::4] = a32[:, sl]
        t = rhs.reshape(KI, 2, N).astype(FP8_NP)
        tl = lhs.reshape(KI, 2, HALF).astype(FP8_NP)

        # Cauchy-Schwarz bound, max over each partition's 4 rows
        cn = np.sqrt(np.cumsum(vt * vt, axis=0))          # [K, N]
        maxn = cn.max(axis=1)                             # [K]
        bound = cn[:, sl] * maxn[:, None] * 1.02          # [K, HALF]
        bq = bound.reshape(K, 128, 4).max(axis=2)         # [K, 128]
        sc = np.ascontiguousarray((127.0 / bq).T)         # [128, K]
        in_maps.append({"t": t, "tl": tl, "sc": sc})
        bounds.append(np.repeat(bq, 4, axis=1))           # [K, HALF]
    return in_maps, bounds


def _assemble(results, bounds) -> np.ndarray:
    out = np.empty((B, K, N, N), dtype=np.float32)
    for c in range(NCORES):
        b, h = divmod(c, 2)
        q = results[c]["out"].astype(np.float32)          # [K, HALF, N]
        q *= (bounds[c] / 127.0)[:, :, None]
        out[b, :, h * HALF:(h + 1) * HALF, :] = q
    return out.reshape(B, K * C, N, N)


def kernel(evecs) -> np.ndarray:
    evecs = np.asarray(evecs, dtype=np.float32)
    assert evecs.shape == (B, C, N, K), evecs.shape
    nc = _get_nc()
    in_maps, bounds = _prepare_in_maps(evecs)
    last_err = None
    for _attempt in range(3):
        try:
            r = run_bass_kernel_spmd(nc, in_maps, list(range(NCORES)))
            return _assemble(r.results, bounds)
        except Exception as e:  # transient NRT/device hiccups: retry
            last_err = e
    raise last_err


# revision 21
# speedup vs baseline: 4.2233x; 1.2252x over previous
"""Trainium2 Bass kernel for nn_ExpandEvecs.

Computes, for evecs [B=4, C=1, N=1024, K=16]:
    cube[b,l] = V[:, :l+1] @ V[:, :l+1]^T   (Gram expansion per level)
    -> [B, K, N, N] fp32 (cumsum of per-eigvec outer products over l).

Sharding: 8 cores = 4 batches x 2 row-halves; core c (b=c//2, h=c%2)
produces all 16 levels for its 512-row half. No communication.

Performance model (per core, 8.4M output elements; all rates HW-measured):
  - The PE streams one 512-column matmul per 427 ns (1.2 GHz sustained;
    the 2.4 GHz p-state needs 3 us of gapless execution, unreachable
    when PSUM drain paces the PE) -> 128 matmuls = 54.7 us. This is the
    kernel's floor: the PE is the only engine that can produce outer
    products at rate (GpSimd tensor ops measured 2.1-15 us per 131K
    elems, DVE fused STT 9.4 us -> offload designs all lose).
  - PSUM evacuation: only ACT (1.2 GHz) and DVE (0.96 GHz) have PSUM
    ports. Whole [128, 2048] ops (one 4-bank PSUM tile) alternate
    between them ~53:47 -> ~35 us in parallel, under the PE floor.
  - int8 output (8.4 MB -> ~23.5 us at the ~358 GB/s per-core HBM
    limit) keeps DMA far off the critical path; fp32 would be 94 us.

Precision (gate 2e-2; simulated end-to-end 4.5e-3):
  - fp8 split matmuls: V = A + B, A = e4m3(V), B = e4m3(V-A);
    V V^T ~= A A^T + A B^T + B A^T (dropped B B^T ~2^-8). 4 stacked
    rows per eigvec k (lhs A,B,A,0 / rhs A,A,B,0) so DoubleRow pairs
    never straddle a k boundary.
  - int8 scale per (level, partition) from the Cauchy-Schwarz bound
    max over the partition's 4 interleaved rows of
    ||v_i||_l * max_j ||v_j||_l (host-computed, 2% margin), applied
    during evacuation (ACT activation scale= / DVE tensor_scalar,
    which round to nearest). Host dequantizes during the unshard.
  - Row-pair interleave: partition p holds DRAM rows 4p..4p+3, giving
    4 KiB contiguous int8 store runs per partition.
"""

import numpy as np
import ml_dtypes

import concourse.mybir as mybir
from concourse import bacc, bass
from concourse.tile import TileContext
from concourse.bass_utils import run_bass_kernel_spmd

B, C, N, K = 4, 1, 1024, 16
NCORES = 8
HALF = N // 2          # rows per core
KI = 2 * K             # DoubleRow pair-partitions at the deepest level

F32 = mybir.dt.float32
FP8 = mybir.dt.float8e4
I8 = mybir.dt.int8
FP8_NP = ml_dtypes.float8_e4m3

_nc_cache = None


def _build():
    nc = bacc.Bacc(None, target_bir_lowering=False)
    t_d = nc.declare_dram_parameter("t", [KI, 2, N], FP8, isOutput=False)
    tl_d = nc.declare_dram_parameter("tl", [KI, 2, HALF], FP8, isOutput=False)
    sc_d = nc.declare_dram_parameter("sc", [128, K * 4], F32, isOutput=False)
    out_d = nc.declare_dram_parameter("out", [K, HALF, N], I8, isOutput=True)

    DR = mybir.MatmulPerfMode.DoubleRow
    COPY = mybir.ActivationFunctionType.Copy
    MUL = mybir.AluOpType.mult
    acc = [0]

    with TileContext(nc) as tc:
        with (
            tc.tile_pool(name="vpool", bufs=1) as vpool,
            tc.tile_pool(name="stage", bufs=3) as stage,
            tc.tile_pool(name="psum", bufs=4, space=bass.MemorySpace.PSUM) as psum,
        ):
            t = vpool.tile([KI, 2, N], FP8)
            tl = vpool.tile([KI, 2, HALF], FP8)
            sc = vpool.tile([128, K * 4], F32)
            t0 = vpool.tile([4, 2, N], FP8)
            tl0 = vpool.tile([4, 2, HALF], FP8)
            # mini slices for levels 0-1 land first and unblock the PE
            # ~1.5us earlier than the full stacks; two HWDGE rings
            nc.sync.dma_start(out=tl0[:], in_=tl_d[:4])
            nc.scalar.dma_start(out=t0[:], in_=t_d[:4])
            nc.sync.dma_start(out=sc[:], in_=sc_d[:])
            nc.sync.dma_start(out=tl[:], in_=tl_d[:])
            nc.scalar.dma_start(out=t[:], in_=t_d[:])

            tlv = tl.rearrange("k o (m r) -> k o m r", m=128, r=4)
            tlv0 = tl0.rearrange("k o (m r) -> k o m r", m=128, r=4)

            for lvl in range(K):
                ki = 2 * (lvl + 1)
                lhs_all, rhs_all = (tlv0, t0) if lvl < 2 else (tlv, t)
                st = stage.tile([128, 4, N], I8, tag="st")
                for r in range(4):
                    ps = psum.tile([128, N], F32, tag="ps")  # 2 banks
                    for j in range(2):
                        nc.tensor.matmul(
                            ps[:, j * 512:(j + 1) * 512],
                            lhsT=lhs_all[:ki, :, :, r],
                            rhs=rhs_all[:ki, :, j * 512:(j + 1) * 512],
                            start=True, stop=True, perf_mode=DR,
                        )
                    # [128, 1024] scale+cast evacuation, alternating
                    # ACT:DVE ~ 8:7 (their measured op-rate ratio)
                    s_ap = sc[:, 4 * lvl + r:4 * lvl + r + 1]
                    acc[0] += 8
                    if acc[0] >= 15:
                        acc[0] -= 15
                        nc.scalar.activation(st[:, r, :], ps[:],
                                             COPY, scale=s_ap)
                    else:
                        nc.vector.tensor_scalar(st[:, r, :],
                                                ps[:], s_ap, None, MUL)
                nc.sync.dma_start(
                    out=out_d[lvl].rearrange("(p r) f -> p r f", p=128),
                    in_=st[:, :, :],
                )

    nc.compile()
    return nc


def _get_nc():
    global _nc_cache
    if _nc_cache is None:
        _nc_cache = _build()
    return _nc_cache


def _prepare_in_maps(evecs: np.ndarray):
    in_maps = []
    bounds = []
    for c in range(NCORES):
        b, h = divmod(c, 2)
        vt = np.ascontiguousarray(evecs[b, 0].T, dtype=np.float32)  # [K, N]
        a32 = vt.astype(FP8_NP).astype(np.float32)
        b32 = (vt - a32).astype(FP8_NP).astype(np.float32)
        sl = slice(h * HALF, (h + 1) * HALF)

        rhs = np.zeros((4 * K, N), dtype=np.float32)
        rhs[0::4] = a32
        rhs[1::4] = a32
        rhs[2::4] = b32
        lhs = np.zeros((4 * K, HALF), dtype=np.float32)
        lhs[0::4] = a32[:, sl]
        lhs[1::4] = b32[:, sl]
        lhs[2::4] = a32[:, sl]
        t = rhs.reshape(KI, 2, N).astype(FP8_NP)
        tl = lhs.reshape(KI, 2, HALF).astype(FP8_NP)

        # Cauchy-Schwarz bound -> per-(level, row) int8 scale, 2% margin
        cn = np.sqrt(np.cumsum(vt * vt, axis=0))          # [K, N]
        maxn = cn.max(axis=1)                             # [K]
        bound = cn[:, sl] * maxn[:, None] * 1.02          # [K, HALF]
        s = (127.0 / bound).astype(np.float32)
        # sc[p, 4*l + r] = s[l, 4p + r]
        sc = np.ascontiguousarray(
            s.reshape(K, 128, 4).transpose(1, 0, 2).reshape(128, K * 4)
        )
        in_maps.append({"t": t, "tl": tl, "sc": sc})
        bounds.append(bound)                              # [K, HALF]
    return in_maps, bounds


def _assemble(results, bounds) -> np.ndarray:
    out = np.empty((B, K, N, N), dtype=np.float32)
    for c in range(NCORES):
        b, h = divmod(c, 2)
        q = results[c]["out"].astype(np.float32)          # [K, HALF, N]
        q *= (bounds[c] / 127.0)[:, :, None]
        out[b, :, h * HALF:(h + 1) * HALF, :] = q
    return out.reshape(B, K * C, N, N)


def kernel(evecs) -> np.ndarray:
    evecs = np.asarray(evecs, dtype=np.float32)
    assert evecs.shape == (B, C, N, K), evecs.shape
    nc = _get_nc()
    in_maps, bounds = _prepare_in_maps(evecs)
    last_err = None
    for _attempt in range(3):
        try:
            r = run_bass_kernel_spmd(nc, in_maps, list(range(NCORES)))
            return _assemble(r.results, bounds)
        except Exception as e:  # transient NRT/device hiccups: retry
            last_err = e
    raise last_err


# revision 25
# speedup vs baseline: 4.2866x; 1.0150x over previous
"""Trainium2 Bass kernel for nn_ExpandEvecs.

Computes, for evecs [B=4, C=1, N=1024, K=16]:
    cube[b,l] = V[:, :l+1] @ V[:, :l+1]^T   (Gram expansion per level)
    -> [B, K, N, N] fp32 (cumsum of per-eigvec outer products over l).

Sharding: 8 cores = 4 batches x 2 row-halves; core c (b=c//2, h=c%2)
produces all 16 levels for its 512-row half. No communication.

Performance model (per core, 8.4M output elements; all rates HW-measured):
  - The PE streams one 512-column matmul per 427 ns (1.2 GHz sustained;
    the 2.4 GHz p-state needs 3 us of gapless execution, unreachable
    when PSUM drain paces the PE) -> 128 matmuls = 54.7 us. This is the
    kernel's floor: the PE is the only engine that can produce outer
    products at rate (GpSimd tensor ops measured 2.1-15 us per 131K
    elems, DVE fused STT 9.4 us -> offload designs all lose).
  - PSUM evacuation: only ACT (1.2 GHz) and DVE (0.96 GHz) have PSUM
    ports. Whole [128, 2048] ops (one 4-bank PSUM tile) alternate
    between them ~53:47 -> ~35 us in parallel, under the PE floor.
  - int8 output (8.4 MB -> ~23.5 us at the ~358 GB/s per-core HBM
    limit) keeps DMA far off the critical path; fp32 would be 94 us.

Precision (gate 2e-2; simulated end-to-end 4.5e-3):
  - fp8 split matmuls: V = A + B, A = e4m3(V), B = e4m3(V-A);
    V V^T ~= A A^T + A B^T + B A^T (dropped B B^T ~2^-8). 4 stacked
    rows per eigvec k (lhs A,B,A,0 / rhs A,A,B,0) so DoubleRow pairs
    never straddle a k boundary.
  - int8 scale per (level, partition) from the Cauchy-Schwarz bound
    max over the partition's 4 interleaved rows of
    ||v_i||_l * max_j ||v_j||_l (host-computed, 2% margin), applied
    during evacuation (ACT activation scale= / DVE tensor_scalar,
    which round to nearest). Host dequantizes during the unshard.
  - Row-pair interleave: partition p holds DRAM rows 4p..4p+3, giving
    4 KiB contiguous int8 store runs per partition.
"""

import numpy as np
import ml_dtypes

import concourse.mybir as mybir
from concourse import bacc, bass
from concourse.tile import TileContext
from concourse.bass_utils import run_bass_kernel_spmd

B, C, N, K = 4, 1, 1024, 16
NCORES = 8
HALF = N // 2          # rows per core
KI = 2 * K             # DoubleRow pair-partitions at the deepest level

F32 = mybir.dt.float32
FP8 = mybir.dt.float8e4
I8 = mybir.dt.int8
FP8_NP = ml_dtypes.float8_e4m3

_nc_cache = None


def _build():
    nc = bacc.Bacc(None, target_bir_lowering=False)
    t_d = nc.declare_dram_parameter("t", [KI, 2, N], FP8, isOutput=False)
    tl_d = nc.declare_dram_parameter("tl", [KI, 2, HALF], FP8, isOutput=False)
    sc_d = nc.declare_dram_parameter("sc", [128, K * 4], F32, isOutput=False)
    out_d = nc.declare_dram_parameter("out", [K, HALF, N], I8, isOutput=True)

    DR = mybir.MatmulPerfMode.DoubleRow
    COPY = mybir.ActivationFunctionType.Copy
    MUL = mybir.AluOpType.mult
    acc = [0]

    with TileContext(nc) as tc:
        with (
            tc.tile_pool(name="vpool", bufs=1) as vpool,
            tc.tile_pool(name="stage", bufs=3) as stage,
            tc.tile_pool(name="psum", bufs=4, space=bass.MemorySpace.PSUM) as psum,
        ):
            t = vpool.tile([KI, 2, N], FP8)
            tl = vpool.tile([KI, 2, HALF], FP8)
            sc = vpool.tile([128, K * 4], F32)
            t0 = vpool.tile([2, 2, N], FP8)
            tl0 = vpool.tile([2, 2, HALF], FP8)
            # tiny level-0 slices land first and unblock the PE earlier
            # than the full stacks; two HWDGE rings
            nc.sync.dma_start(out=tl0[:], in_=tl_d[:2])
            nc.scalar.dma_start(out=t0[:], in_=t_d[:2])
            nc.sync.dma_start(out=sc[:], in_=sc_d[:])
            nc.sync.dma_start(out=tl[:], in_=tl_d[:])
            nc.scalar.dma_start(out=t[:], in_=t_d[:])

            tlv = tl.rearrange("k o (m r) -> k o m r", m=128, r=4)
            tlv0 = tl0.rearrange("k o (m r) -> k o m r", m=128, r=4)

            for lvl in range(K):
                ki = 2 * (lvl + 1)
                lhs_all, rhs_all = (tlv0, t0) if lvl < 1 else (tlv, t)
                tail = lvl == K - 1
                st = stage.tile([128, 4, N], I8, tag="st")
                for r in range(4):
                    ps = psum.tile([128, N], F32, tag="ps")  # 2 banks
                    for j in range(2):
                        nc.tensor.matmul(
                            ps[:, j * 512:(j + 1) * 512],
                            lhsT=lhs_all[:ki, :, :, r],
                            rhs=rhs_all[:ki, :, j * 512:(j + 1) * 512],
                            start=True, stop=True, perf_mode=DR,
                        )
                    s_ap = sc[:, 4 * lvl + r:4 * lvl + r + 1]
                    if tail:
                        # pipeline drain: both engines in parallel on the
                        # tile's two banks, store each r-slice immediately
                        nc.scalar.activation(st[:, r, :512], ps[:, :512],
                                             COPY, scale=s_ap)
                        nc.vector.tensor_scalar(st[:, r, 512:],
                                                ps[:, 512:], s_ap, None, MUL)
                        nc.sync.dma_start(
                            out=out_d[lvl].rearrange(
                                "(p r) f -> p r f", p=128)[:, r, :],
                            in_=st[:, r, :])
                        continue
                    # [128, 1024] scale+cast evacuation, alternating
                    # ACT:DVE ~ 8:7 (their measured op-rate ratio)
                    acc[0] += 8
                    if acc[0] >= 15:
                        acc[0] -= 15
                        nc.scalar.activation(st[:, r, :], ps[:],
                                             COPY, scale=s_ap)
                    else:
                        nc.vector.tensor_scalar(st[:, r, :],
                                                ps[:], s_ap, None, MUL)
                if not tail:
                    nc.sync.dma_start(
                        out=out_d[lvl].rearrange("(p r) f -> p r f", p=128),
                        in_=st[:, :, :],
                    )

    nc.compile()
    return nc


def _get_nc():
    global _nc_cache
    if _nc_cache is None:
        _nc_cache = _build()
    return _nc_cache


def _prepare_in_maps(evecs: np.ndarray):
    in_maps = []
    bounds = []
    for c in range(NCORES):
        b, h = divmod(c, 2)
        vt = np.ascontiguousarray(evecs[b, 0].T, dtype=np.float32)  # [K, N]
        a32 = vt.astype(FP8_NP).astype(np.float32)
        b32 = (vt - a32).astype(FP8_NP).astype(np.float32)
        sl = slice(h * HALF, (h + 1) * HALF)

        rhs = np.zeros((4 * K, N), dtype=np.float32)
        rhs[0::4] = a32
        rhs[1::4] = a32
        rhs[2::4] = b32
        lhs = np.zeros((4 * K, HALF), dtype=np.float32)
        lhs[0::4] = a32[:, sl]
        lhs[1::4] = b32[:, sl]
        lhs[2::4] = a32[:, sl]
        t = rhs.reshape(KI, 2, N).astype(FP8_NP)
        tl = lhs.reshape(KI, 2, HALF).astype(FP8_NP)

        # Cauchy-Schwarz bound -> per-(level, row) int8 scale, 2% margin
        cn = np.sqrt(np.cumsum(vt * vt, axis=0))          # [K, N]
        maxn = cn.max(axis=1)                             # [K]
        bound = cn[:, sl] * maxn[:, None] * 1.02          # [K, HALF]
        s = (127.0 / bound).astype(np.float32)
        # sc[p, 4*l + r] = s[l, 4p + r]
        sc = np.ascontiguousarray(
            s.reshape(K, 128, 4).transpose(1, 0, 2).reshape(128, K * 4)
        )
        in_maps.append({"t": t, "tl": tl, "sc": sc})
        bounds.append(bound)                              # [K, HALF]
    return in_maps, bounds


def _assemble(results, bounds) -> np.ndarray:
    out = np.empty((B, K, N, N), dtype=np.float32)
    for c in range(NCORES):
        b, h = divmod(c, 2)
        q = results[c]["out"].astype(np.float32)          # [K, HALF, N]
        q *= (bounds[c] / 127.0)[:, :, None]
        out[b, :, h * HALF:(h + 1) * HALF, :] = q
    return out.reshape(B, K * C, N, N)


def kernel(evecs) -> np.ndarray:
    evecs = np.asarray(evecs, dtype=np.float32)
    assert evecs.shape == (B, C, N, K), evecs.shape
    nc = _get_nc()
    in_maps, bounds = _prepare_in_maps(evecs)
    last_err = None
    for _attempt in range(3):
        try:
            r = run_bass_kernel_spmd(nc, in_maps, list(range(NCORES)))
            return _assemble(r.results, bounds)
        except Exception as e:  # transient NRT/device hiccups: retry
            last_err = e
    raise last_err
